# Initial kernel scaffold
#
"""Causal single-head attention (B=4, S=4096, E=1024, H=128) on 8 TRN2 NeuronCores.

Sharding: 8 cores = 4 batches x 2 sequence shards. Each core handles 4 query
blocks of 512 rows of one batch. Causal work per q-block j is 4*(j+1) k-tiles
(128 keys each); blocks are split {7,5,2,0} / {6,4,3,1} so both shards cost 72
k-tiles, padded to a uniform program of [32,24,16,8] k-tiles per slot so all 8
cores run one SPMD program. Per-core data (gathered Q columns + causal masks)
encodes which q-blocks a core owns.

Per core: project Q^T, K^T, V^T from embds^T (bf16 matmuls, fp32 PSUM),
transpose V^T->V on the PE, then flash-style attention in scores-transposed
layout (scores^T = K_tile^T.T @ Q^T), exp on ACT, multiplicative causal masks,
AV accumulated in PSUM over k-tiles, and ones-matmul row-sums for the softmax
normalizer (reciprocal + DMA partition-broadcast at the end).
"""

import numpy as np
import ml_dtypes

import concourse.bacc as bacc
import concourse.bass as bass
import concourse.mybir as mybir
import concourse.tile as tile
from concourse.bass_utils import run_bass_kernel_spmd
from concourse.masks import make_identity

BF16 = ml_dtypes.bfloat16
F32 = np.float32

B, S, E, H = 4, 4096, 1024, 128
NCORES = 8
PROG = [32, 24, 16, 8]                       # program k-tile count per slot
BLOCKS = {0: [7, 5, 2, 0], 1: [6, 4, 3, 1]}  # parity -> owned q-block ids
EC = E // 128                                 # 8 contraction chunks
SB = S // 512                                 # 8 key blocks of 512
QB = 4                                        # q-blocks (slots) per core
QLEN = QB * 512                               # 2048 q rows per core

USE_F32R_DENOM = True                         # pacc in f32r for 1cyc/row L-matmuls

_CACHE = {}


def _build_program():
    dt = mybir.dt
    nc = bacc.Bacc("TRN2", target_bir_lowering=False, debug=False, num_devices=NCORES)

    embT_d = nc.dram_tensor("embT", [E, S], dt.bfloat16, kind="ExternalInput")
    embTq_d = nc.dram_tensor("embTq", [E, QLEN], dt.bfloat16, kind="ExternalInput")
    wq_d = nc.dram_tensor("wq", [E, H], dt.bfloat16, kind="ExternalInput")
    wk_d = nc.dram_tensor("wk", [E, H], dt.bfloat16, kind="ExternalInput")
    wv_d = nc.dram_tensor("wv", [E, H], dt.bfloat16, kind="ExternalInput")
    bq_d = nc.dram_tensor("bq", [H, 1], dt.float32, kind="ExternalInput")
    bk_d = nc.dram_tensor("bk", [H, 1], dt.float32, kind="ExternalInput")
    bv_d = nc.dram_tensor("bv", [H, 1], dt.float32, kind="ExternalInput")
    mask_d = nc.dram_tensor("maskblk", [QB, 128, 8, 512], dt.bfloat16, kind="ExternalInput")
    out_d = nc.dram_tensor("out", [QLEN, H], dt.float32, kind="ExternalOutput")

    ident_f = mybir.ActivationFunctionType.Identity
    exp_f = mybir.ActivationFunctionType.Exp
    dn_dt = dt.float32r if USE_F32R_DENOM else dt.float32

    with tile.TileContext(nc) as tc:
        with tc.tile_pool(name="singles", bufs=1) as singles:
            # ---- constants: weights/biases first on the SP queue (startup path) ----
            w_sb = {}
            b_sb = {}
            for name, wd, bd in (("k", wk_d, bk_d), ("v", wv_d, bv_d), ("q", wq_d, bq_d)):
                w = singles.tile([128, EC, H], dt.bfloat16, tag=f"w{name}", name=f"w{name}")
                nc.sync.dma_start(out=w[:, :, :], in_=wd.ap().rearrange("(c p) h -> p c h", p=128))
                w_sb[name] = w
                b = singles.tile([H, 1], dt.float32, tag=f"b{name}", name=f"b{name}")
                nc.sync.dma_start(out=b[:, :], in_=bd.ap())
                b_sb[name] = b
            identb = singles.tile([128, 128], dt.bfloat16, tag="identb")
            make_identity(nc, identb[:, :])
            identf = singles.tile([128, 128], dt.float32, tag="identf")
            make_identity(nc, identf[:, :])
            ones_f32 = singles.tile([128, 1], dt.float32, tag="ones_f32")
            nc.vector.memset(ones_f32[:, :], 1.0)
            if USE_F32R_DENOM:
                ones_col = singles.tile([128, 1], dn_dt, tag="ones_col")
                nc.vector.tensor_copy(ones_col[:, :], ones_f32[:, :])
            else:
                ones_col = ones_f32
            ones_row = singles.tile([1, 128], dt.float32, tag="ones_row")
            nc.vector.memset(ones_row[:, :], 1.0)

            # per-block tensors, split for fine-grained dependencies
            qTs = [singles.tile([128, 512], dt.bfloat16, tag=f"qT{i}", name=f"qT{i}") for i in range(QB)]
            kTs = [singles.tile([128, 512], dt.bfloat16, tag=f"kT{i}", name=f"kT{i}") for i in range(SB)]
            vTs = [singles.tile([128, 512], dt.bfloat16, tag=f"vT{i}", name=f"vT{i}") for i in range(SB)]
            vts = [singles.tile([128, 128], dt.bfloat16, tag=f"v{i}", name=f"v{i}") for i in range(S // 128)]

            def kv_block(sb, etp, pkp, pvp, ptrvp):
                et = etp.tile([128, EC, 512], dt.bfloat16, tag="et", name=f"et{sb}")
                for c2 in range(EC // 2):
                    eng = nc.sync if (c2 % 2 == 0) else nc.scalar
                    eng.dma_start(
                        out=et[:, 2 * c2:2 * c2 + 2, :],
                        in_=embT_d.ap()
                        .rearrange("(cc c p) s -> p cc c s", c=2, p=128)[:, c2, :, 512 * sb:512 * (sb + 1)],
                    )
                psk = pkp.tile([128, 512], dt.float32, tag="psk", name=f"psk{sb}")
                psv = pvp.tile([128, 512], dt.float32, tag="psv", name=f"psv{sb}")
                for c in range(EC):
                    nc.tensor.matmul(psk[:, :], lhsT=w_sb["k"][:, c, :], rhs=et[:, c, :],
                                     start=(c == 0), stop=(c == EC - 1))
                    nc.tensor.matmul(psv[:, :], lhsT=w_sb["v"][:, c, :], rhs=et[:, c, :],
                                     start=(c == 0), stop=(c == EC - 1))
                nc.scalar.activation(kTs[sb][:, :], psk[:, :], ident_f, bias=b_sb["k"][:, :])
                nc.scalar.activation(vTs[sb][:, :], psv[:, :], ident_f, bias=b_sb["v"][:, :])
                for u in range(4):
                    st = 4 * sb + u
                    tp = ptrvp.tile([128, 128], dt.bfloat16, tag="tp", name=f"tp{st}")
                    nc.tensor.transpose(tp[:, :], vTs[sb][:, 128 * u:128 * (u + 1)], identb[:, :])
                    nc.any.tensor_copy(vts[st][:, :], tp[:, :])

            with tc.tile_pool(name="et", bufs=4) as etp, \
                 tc.tile_pool(name="pk", bufs=2, space="PSUM") as pkp, \
                 tc.tile_pool(name="pv", bufs=2, space="PSUM") as pvp, \
                 tc.tile_pool(name="ptrv", bufs=2, space="PSUM") as ptrvp:
                kv_block(0, etp, pkp, pvp, ptrvp)
                # Q^T projection (attention slot 0 needs qT[0] + kT[0] first)
                for qb2 in range(QB // 2):
                    etq = etp.tile([128, EC, 1024], dt.bfloat16, tag="etq", name=f"etq{qb2}")
                    for c in range(EC):
                        nc.scalar.dma_start(
                            out=etq[:, c, :],
                            in_=embTq_d.ap()[128 * c:128 * (c + 1), 1024 * qb2:1024 * (qb2 + 1)],
                        )
                    for h2 in (0, 1):
                        qb = 2 * qb2 + h2
                        psq = pkp.tile([128, 512], dt.float32, tag="psk", name=f"psq{qb}")
                        for c in range(EC):
                            nc.tensor.matmul(psq[:, :], lhsT=w_sb["q"][:, c, :],
                                             rhs=etq[:, c, 512 * h2:512 * (h2 + 1)],
                                             start=(c == 0), stop=(c == EC - 1))
                        nc.scalar.activation(qTs[qb][:, :], psq[:, :], ident_f, bias=b_sb["q"][:, :])
                for sb in range(1, SB):
                    kv_block(sb, etp, pkp, pvp, ptrvp)

            # ---- attention ----
            with tc.tile_pool(name="ps", bufs=2, space="PSUM") as psp, \
                 tc.tile_pool(name="po", bufs=2, space="PSUM") as pop, \
                 tc.tile_pool(name="pl", bufs=1, space="PSUM") as plp, \
                 tc.tile_pool(name="ptro", bufs=1, space="PSUM") as ptrop, \
                 tc.tile_pool(name="ptbuf", bufs=3) as ptp, \
                 tc.tile_pool(name="mask", bufs=2) as mkp, \
                 tc.tile_pool(name="pacc", bufs=2) as paccp, \
                 tc.tile_pool(name="ep", bufs=2) as epp:
                for s in range(QB):
                    Wp = PROG[s]
                    mt = mkp.tile([128, 8, 512], dt.bfloat16, tag="mt", name=f"mt{s}")
                    nc.sync.dma_start(out=mt[:, :, :], in_=mask_d.ap()[s])
                    po = pop.tile([128, 512], dt.float32, tag="po", name=f"po{s}")
                    pacc_a = paccp.tile([128, 512], dn_dt, tag="pacc_a", name=f"pacc_a{s}")
                    pacc_b = paccp.tile([128, 512], dn_dt, tag="pacc_b", name=f"pacc_b{s}")
                    for p in range(Wp // 2):
                        ps = psp.tile([128, 2, 512], dt.float32, tag="ps", name=f"ps{s}_{p}")
                        for h2 in (0, 1):
                            t = 2 * p + h2
                            nc.tensor.matmul(ps[:, h2, :],
                                             lhsT=kTs[t // 4][:, 128 * (t % 4):128 * (t % 4 + 1)],
                                             rhs=qTs[s][:, :], start=True, stop=True)
                        pt = ptp.tile([128, 2, 512], dt.bfloat16, tag="pt", name=f"pt{s}_{p}")
                        nc.scalar.activation(pt[:, :, :], ps[:, :, :], exp_f)
                        for h2 in (0, 1):
                            t = 2 * p + h2
                            if t >= Wp - 8:
                                nc.vector.tensor_mul(pt[:, h2, :], pt[:, h2, :],
                                                     mt[:, t - (Wp - 8), :])
                        # softmax denominator partials: even k-tiles on DVE, odd on GPSIMD
                        if p == 0:
                            nc.vector.tensor_copy(pacc_a[:, :], pt[:, 0, :])
                            nc.gpsimd.tensor_copy(pacc_b[:, :], pt[:, 1, :])
                        else:
                            nc.vector.tensor_add(pacc_a[:, :], pacc_a[:, :], pt[:, 0, :])
                            nc.gpsimd.tensor_add(pacc_b[:, :], pacc_b[:, :], pt[:, 1, :])
                        for h2 in (0, 1):
                            t = 2 * p + h2
                            nc.tensor.matmul(po[:, :], lhsT=vts[t][:, :],
                                             rhs=pt[:, h2, :], start=(t == 0), stop=(t == Wp - 1))
                    # epilogue: L = colsum(pacc_a + pacc_b); out = (po / L).T
                    pl = plp.tile([1, 512], dt.float32, tag="pl", name=f"pl{s}")
                    nc.tensor.matmul(pl[:, :], lhsT=ones_col[:, :], rhs=pacc_a[:, :],
                                     start=True, stop=False)
                    nc.tensor.matmul(pl[:, :], lhsT=ones_col[:, :], rhs=pacc_b[:, :],
                                     start=False, stop=True)
                    recip = epp.tile([1, 512], dt.float32, tag="recip", name=f"recip{s}")
                    nc.vector.reciprocal(recip[:, :], pl[:, :])
                    pb = plp.tile([128, 512], dt.float32, tag="pl", name=f"pb{s}")
                    nc.tensor.matmul(pb[:, :], lhsT=ones_row[:, :], rhs=recip[:, :],
                                     start=True, stop=True)
                    rb_sb = epp.tile([128, 512], dt.float32, tag="rb_sb", name=f"rb{s}")
                    nc.vector.tensor_copy(rb_sb[:, :], pb[:, :])
                    onrm = epp.tile([128, 512], dt.float32, tag="onrm", name=f"onrm{s}")
                    nc.vector.tensor_mul(onrm[:, :], po[:, :], rb_sb[:, :])
                    oc = epp.tile([128, 4, 128], dt.float32, tag="oc", name=f"oc{s}")
                    for u in range(4):
                        tp2 = ptrop.tile([128, 128], dt.float32, tag="tp2", name=f"tp2_{s}_{u}")
                        nc.tensor.transpose(tp2[:, :], onrm[:, 128 * u:128 * (u + 1)],
                                            identf[:, :])
                        nc.vector.tensor_copy(oc[:, u, :], tp2[:, :])
                    nc.sync.dma_start(
                        out=out_d.ap()[512 * s:512 * (s + 1), :].rearrange("(u p) h -> p u h", p=128),
                        in_=oc[:, :, :],
                    )

    nc.compile()
    return nc


def _build_maskblk(parity):
    m = np.zeros((QB, 128, 8, 512), np.float32)
    kk = np.arange(128)[:, None]
    qq = np.arange(512)[None, :]
    for s, j in enumerate(BLOCKS[parity]):
        Wp, Wa = PROG[s], 4 * (j + 1)
        for i in range(8):
            d = (Wp - 8 + i) - (Wa - 4)
            m[s, :, i, :] = ((qq - 128 * d) >= kk)
    return m.astype(BF16)


def kernel(embds, Wq, bq, Wk, bk, Wv, bv):
    embds = np.asarray(embds, F32)
    Wq = np.asarray(Wq, F32); bq = np.asarray(bq, F32)
    Wk = np.asarray(Wk, F32); bk = np.asarray(bk, F32)
    Wv = np.asarray(Wv, F32); bv = np.asarray(bv, F32)

    if "nc" not in _CACHE:
        _CACHE["nc"] = _build_program()
    nc = _CACHE["nc"]

    scale = F32(1.0 / np.sqrt(H))
    wq_h = (Wq * scale).astype(BF16)
    wk_h = Wk.astype(BF16)
    wv_h = Wv.astype(BF16)
    bq_h = (bq * scale).astype(F32).reshape(H, 1)
    bk_h = bk.astype(F32).reshape(H, 1)
    bv_h = bv.astype(F32).reshape(H, 1)
    masks = {p: _build_maskblk(p) for p in (0, 1)}

    embT = {b: np.ascontiguousarray(embds[b].T).astype(BF16) for b in range(B)}

    in_maps = []
    for c in range(NCORES):
        b, parity = c // 2, c % 2
        et = embT[b]
        etq = np.concatenate([et[:, 512 * j:512 * (j + 1)] for j in BLOCKS[parity]], axis=1)
        in_maps.append({
            "embT": et,
            "embTq": np.ascontiguousarray(etq),
            "wq": wq_h, "wk": wk_h, "wv": wv_h,
            "bq": bq_h, "bk": bk_h, "bv": bv_h,
            "maskblk": masks[parity],
        })

    res = run_bass_kernel_spmd(nc, in_maps, list(range(NCORES)))

    out = np.empty((B, S, H), F32)
    for c in range(NCORES):
        b, parity = c // 2, c % 2
        oc = res.results[c]["out"]
        for s, j in enumerate(BLOCKS[parity]):
            out[b, 512 * j:512 * (j + 1)] = oc[512 * s:512 * (s + 1)]
    return out



# revision 6
# speedup vs baseline: 1.2312x; 1.2312x over previous
"""Causal single-head attention (B=4, S=4096, E=1024, H=128) on 8 TRN2 NeuronCores.

Sharding: 8 cores = 4 batches x 2 sequence shards. Each core handles 4 query
blocks of 512 rows of one batch. Causal work per q-block j is 4*(j+1) k-tiles
(128 keys each); blocks are split {7,5,2,0} / {6,4,3,1} so both shards cost 72
k-tiles, padded to a uniform program of [32,24,16,8] k-tiles per slot so all 8
cores run one SPMD program.

Per core, one interleaved pipeline:
  - K^T/V^T projection blocks (bf16 matmuls, fp32 PSUM) streamed off chunked
    embT DMAs, V transposed on the PE, followed by flash-style attention in
    scores-transposed layout (scores^T = K_tile^T.T @ Q^T), exp on ACT,
    causal masking via fused DVE tensor_paged_mask driven by a tiny per-core
    threshold tensor (no mask DMA), AV accumulated in PSUM.
  - Q^T projections and output epilogues are emitted between attention pairs
    as PE filler work so the tensor engine never idles.
  - Softmax denominator: DVE/GPSIMD accumulate exp partials; tiny
    [128,1]-output matmuls reduce them per 128-column chunk into column form,
    so normalization is a per-partition scalar multiply fused with the
    PSUM->SBUF copy after the PE output transpose.
  - ~3us of tiny warm-up matmuls at t=0 keep the PE p-state ramp warm through
    the first DMA latency.
"""

import numpy as np
import ml_dtypes

import concourse.bacc as bacc
import concourse.bass as bass
import concourse.mybir as mybir
import concourse.tile as tile
from concourse.bass_utils import run_bass_kernel_spmd
from concourse.masks import make_identity

BF16 = ml_dtypes.bfloat16
F32 = np.float32

B, S, E, H = 4, 4096, 1024, 128
NCORES = 8
PROG = [32, 24, 16, 8]                       # program k-tile count per slot
BLOCKS = {0: [7, 5, 2, 0], 1: [6, 4, 3, 1]}  # parity -> owned q-block ids
EC = E // 128                                 # 8 contraction chunks
SB = S // 512                                 # 8 key blocks of 512
QB = 4                                        # q-blocks (slots) per core
QLEN = QB * 512                               # 2048 q rows per core

_CACHE = {}


def _build_program():
    dt = mybir.dt
    nc = bacc.Bacc("TRN2", target_bir_lowering=False, debug=False, num_devices=NCORES)

    embT_d = nc.dram_tensor("embT", [E, S], dt.bfloat16, kind="ExternalInput")
    embTq_d = nc.dram_tensor("embTq", [E, QLEN], dt.bfloat16, kind="ExternalInput")
    wpack_d = nc.dram_tensor("wpack", [128, 3, EC, H], dt.bfloat16, kind="ExternalInput")
    bpack_d = nc.dram_tensor("bpack", [128, 3], dt.float32, kind="ExternalInput")
    thresh_d = nc.dram_tensor("thresh", [128, 16], dt.float32, kind="ExternalInput")
    out_d = nc.dram_tensor("out", [QLEN, H], dt.float32, kind="ExternalOutput")

    ident_f = mybir.ActivationFunctionType.Identity
    exp_f = mybir.ActivationFunctionType.Exp

    with tile.TileContext(nc) as tc:
        with tc.tile_pool(name="singles", bufs=1) as singles, \
             tc.tile_pool(name="etp", bufs=3) as etp, \
             tc.tile_pool(name="qetp", bufs=2) as qetp, \
             tc.tile_pool(name="vtp", bufs=2) as vtp, \
             tc.tile_pool(name="ptp", bufs=3) as ptp, \
             tc.tile_pool(name="paccp", bufs=2) as paccp, \
             tc.tile_pool(name="ocp", bufs=2) as ocp, \
             tc.tile_pool(name="posp", bufs=2) as posp, \
             tc.tile_pool(name="recp", bufs=2) as recp, \
             tc.tile_pool(name="psp", bufs=2, space="PSUM") as psp, \
             tc.tile_pool(name="pop", bufs=2, space="PSUM") as pop, \
             tc.tile_pool(name="auxp", bufs=2, space="PSUM") as auxp:

            # ---- constants / weights (front of the SP DMA queue) ----
            wpack = singles.tile([128, 3, EC, H], dt.bfloat16, tag="wpack")
            nc.sync.dma_start(out=wpack[:, 0, :, :], in_=wpack_d.ap()[:, 0])
            bp = singles.tile([128, 3], dt.float32, tag="bp")
            nc.sync.dma_start(out=bp[:, :], in_=bpack_d.ap())
            th = singles.tile([128, 16], dt.float32, tag="th")
            nc.sync.dma_start(out=th[:, :], in_=thresh_d.ap())

            identb = singles.tile([128, 128], dt.bfloat16, tag="identb")
            make_identity(nc, identb[:, :])
            identf = singles.tile([128, 128], dt.float32, tag="identf")
            make_identity(nc, identf[:, :])
            ones_f = singles.tile([128, 1], dt.float32, tag="ones_f")
            nc.vector.memset(ones_f[:, :], 1.0)
            ones_r = singles.tile([128, 1], dt.float32r, tag="ones_r")
            nc.vector.tensor_copy(ones_r[:, :], ones_f[:, :])
            # paged-mask offsets: offs[p, j, q] = q + 1 (fp16 keeps DVE 2x mode)
            offs = singles.tile([128, 2, 512], dt.float16, tag="offs")
            nc.gpsimd.iota(offs[:, :, :], pattern=[[0, 2], [1, 512]], base=1,
                           channel_multiplier=0, allow_small_or_imprecise_dtypes=True)

            # ---- PE warm-up: keep the p-state ramp alive through DMA latency ----
            for i in range(55):
                wm = auxp.tile([128, 64], dt.float32, tag="aux", name=f"wm{i}")
                nc.tensor.matmul(wm[:, :], lhsT=identb[:, :], rhs=identb[:, :64],
                                 start=True, stop=True)

            # remaining weights (V, Q) behind the first et chunks
            nc.sync.dma_start(out=wpack[:, 1:3, :, :], in_=wpack_d.ap()[:, 1:3])

            kTs = [singles.tile([128, 512], dt.bfloat16, tag=f"kT{i}", name=f"kT{i}")
                   for i in range(SB)]
            vts = [singles.tile([128, 128], dt.bfloat16, tag=f"v{i}", name=f"v{i}")
                   for i in range(4 * SB)]
            qTs = [singles.tile([128, 512], dt.bfloat16, tag=f"qT{i}", name=f"qT{i}")
                   for i in range(QB)]

            pending_tr = []  # deferred V-transposes: (st, vT tile, u)

            def flush_tr(n=1000):
                # one PE transpose + DVE copy per call site; spaced so the
                # 2-slot aux rotation never stalls the PE on the DVE copy
                for _ in range(min(n, len(pending_tr))):
                    st, vT, u = pending_tr.pop(0)
                    tp = auxp.tile([128, 128], dt.bfloat16, tag="aux", name=f"tp{st}")
                    nc.tensor.transpose(tp[:, :], vT[:, 128 * u:128 * (u + 1)],
                                        identb[:, :])
                    nc.vector.tensor_copy(vts[st][:, :], tp[:, :])

            def kv_block(sb):
                et = etp.tile([128, EC, 512], dt.bfloat16, tag="et", name=f"et{sb}")
                for c2 in range(EC // 2):
                    nc.sync.dma_start(
                        out=et[:, 2 * c2:2 * c2 + 2, :],
                        in_=embT_d.ap()
                        .rearrange("(cc c p) s -> p cc c s", c=2, p=128)[:, c2, :, 512 * sb:512 * (sb + 1)],
                    )
                ps = psp.tile([128, 2, 512], dt.float32, tag="ps", name=f"pskv{sb}")
                for c in range(EC):
                    nc.tensor.matmul(ps[:, 0, :], lhsT=wpack[:, 0, c, :], rhs=et[:, c, :],
                                     start=(c == 0), stop=(c == EC - 1))
                    nc.tensor.matmul(ps[:, 1, :], lhsT=wpack[:, 1, c, :], rhs=et[:, c, :],
                                     start=(c == 0), stop=(c == EC - 1))
                    if c >= 3:
                        flush_tr(1)  # previous block's V transposes, one per chunk
                nc.scalar.activation(kTs[sb][:, :], ps[:, 0, :], ident_f, bias=bp[:, 0:1])
                vT = vtp.tile([128, 512], dt.bfloat16, tag="vT", name=f"vT{sb}")
                nc.scalar.activation(vT[:, :], ps[:, 1, :], ident_f, bias=bp[:, 1:2])
                pending_tr.extend((4 * sb + u, vT, u) for u in range(4))

            # filler units: emitted between attention pairs to keep PE dense
            fillers = []

            def emit_fillers(n):
                for _ in range(n):
                    if fillers:
                        fillers.pop(0)()

            def qproj(s):
                qet = qetp.tile([128, EC, 512], dt.bfloat16, tag="qet", name=f"qet{s}")
                for c2 in range(EC // 2):
                    nc.sync.dma_start(
                        out=qet[:, 2 * c2:2 * c2 + 2, :],
                        in_=embTq_d.ap()
                        .rearrange("(cc c p) s -> p cc c s", c=2, p=128)[:, c2, :, 512 * s:512 * (s + 1)],
                    )
                ps = psp.tile([128, 2, 512], dt.float32, tag="ps", name=f"psq{s}")
                for c in range(EC):
                    nc.tensor.matmul(ps[:, 0, :], lhsT=wpack[:, 2, c, :],
                                     rhs=qet[:, c, :],
                                     start=(c == 0), stop=(c == EC - 1))
                nc.scalar.activation(qTs[s][:, :], ps[:, 0, :], ident_f,
                                     bias=bp[:, 2:3])

            def att_slot(s):
                Wp = PROG[s]
                po = pop.tile([128, 512], dt.float32, tag="po", name=f"po{s}")
                pacc_a = paccp.tile([128, 512], dt.float32r, tag="pacc_a", name=f"pacc_a{s}")
                pacc_b = paccp.tile([128, 512], dt.float32r, tag="pacc_b", name=f"pacc_b{s}")
                for p in range(Wp // 2):
                    ps = psp.tile([128, 2, 512], dt.float32, tag="ps", name=f"ps{s}_{p}")
                    for h2 in (0, 1):
                        t = 2 * p + h2
                        nc.tensor.matmul(ps[:, h2, :],
                                         lhsT=kTs[t // 4][:, 128 * (t % 4):128 * (t % 4 + 1)],
                                         rhs=qTs[s][:, :], start=True, stop=True)
                    pt = ptp.tile([128, 2, 512], dt.bfloat16, tag="pt", name=f"pt{s}_{p}")
                    nc.scalar.activation(pt[:, :, :], ps[:, :, :], exp_f)
                    pp = p - (Wp // 2 - 4)
                    if pp >= 0:
                        # fused causal mask: keep iff th[k] + 128*sub < q+1
                        nc.vector.tensor_paged_mask(
                            pt[:, :, :], pt[:, :, :],
                            partition_indices=th[:, 4 * s + pp:4 * s + pp + 1],
                            partition_step=128.0,
                            mask_offsets=offs[:, :, :],
                        )
                    if p == 0:
                        nc.vector.tensor_copy(pacc_a[:, :], pt[:, 0, :])
                        nc.gpsimd.tensor_copy(pacc_b[:, :], pt[:, 1, :])
                    else:
                        nc.vector.tensor_add(pacc_a[:, :], pacc_a[:, :], pt[:, 0, :])
                        nc.gpsimd.tensor_add(pacc_b[:, :], pacc_b[:, :], pt[:, 1, :])
                    for h2 in (0, 1):
                        t = 2 * p + h2
                        nc.tensor.matmul(po[:, :], lhsT=vts[t][:, :], rhs=pt[:, h2, :],
                                         start=(t == 0), stop=(t == Wp - 1))
                    emit_fillers(1)
                # ---- epilogue head: column-form denominator (inline, cheap) ----
                lc = auxp.tile([128, 4], dt.float32, tag="aux", name=f"lc{s}")
                for u in range(4):
                    nc.tensor.matmul(lc[:, u:u + 1], lhsT=pacc_a[:, 128 * u:128 * (u + 1)],
                                     rhs=ones_r[:, :], start=True, stop=False)
                    nc.tensor.matmul(lc[:, u:u + 1], lhsT=pacc_b[:, 128 * u:128 * (u + 1)],
                                     rhs=ones_r[:, :], start=False, stop=True)
                rec = recp.tile([128, 4], dt.float32, tag="rec", name=f"rec{s}")
                nc.vector.reciprocal(rec[:, :], lc[:, :])
                pos = posp.tile([128, 512], dt.float32, tag="pos", name=f"pos{s}")
                nc.vector.tensor_copy(pos[:, :], po[:, :])
                # ---- epilogue tail: transpose + normalize-on-copy, as fillers ----
                oc = ocp.tile([128, 4, 128], dt.float32, tag="oc", name=f"oc{s}")

                def tr_piece(u):
                    def emit():
                        tro = auxp.tile([128, 128], dt.float32, tag="aux",
                                        name=f"tro{s}_{u}")
                        nc.tensor.transpose(tro[:, :], pos[:, 128 * u:128 * (u + 1)],
                                            identf[:, :])
                        nc.vector.tensor_scalar_mul(oc[:, u, :], tro[:, :], rec[:, u:u + 1])
                        if u == 3:
                            nc.sync.dma_start(
                                out=out_d.ap()[512 * s:512 * (s + 1), :]
                                .rearrange("(u p) h -> p u h", p=128),
                                in_=oc[:, :, :],
                            )
                    return emit

                fillers.extend(tr_piece(u) for u in range(4))

            # ---- pipeline ----
            for sb in range(SB):
                kv_block(sb)
            qproj(3)
            flush_tr()
            att_slot(3)
            qproj(2)
            att_slot(2)
            qproj(1)
            att_slot(1)
            qproj(0)
            att_slot(0)
            emit_fillers(len(fillers))

    nc.compile()
    return nc


def _build_thresh(parity):
    t = np.zeros((128, 16), np.float32)
    kk = np.arange(128, dtype=np.float32)
    for s, j in enumerate(BLOCKS[parity]):
        Wp, Wa = PROG[s], 4 * (j + 1)
        for pp in range(4):
            t0 = Wp - 8 + 2 * pp
            d0 = t0 - (Wa - 4)
            t[:, 4 * s + pp] = kk + 128.0 * d0
    return t


def kernel(embds, Wq, bq, Wk, bk, Wv, bv):
    embds = np.asarray(embds, F32)
    Wq = np.asarray(Wq, F32); bq = np.asarray(bq, F32)
    Wk = np.asarray(Wk, F32); bk = np.asarray(bk, F32)
    Wv = np.asarray(Wv, F32); bv = np.asarray(bv, F32)

    if "nc" not in _CACHE:
        _CACHE["nc"] = _build_program()
    nc = _CACHE["nc"]

    scale = F32(1.0 / np.sqrt(H))

    def to_lhsT(w):
        return np.ascontiguousarray(
            w.astype(BF16).reshape(EC, 128, H).transpose(1, 0, 2))

    wpack = np.ascontiguousarray(
        np.stack([to_lhsT(Wk), to_lhsT(Wv), to_lhsT(Wq * scale)], axis=1))
    bpack = np.ascontiguousarray(
        np.stack([bk, bv, bq * scale], axis=1).astype(F32))
    thresh = {p: _build_thresh(p) for p in (0, 1)}

    embT = {b: np.ascontiguousarray(embds[b].T).astype(BF16) for b in range(B)}

    in_maps = []
    for c in range(NCORES):
        b, parity = c // 2, c % 2
        et = embT[b]
        etq = np.concatenate([et[:, 512 * j:512 * (j + 1)] for j in BLOCKS[parity]], axis=1)
        in_maps.append({
            "embT": et,
            "embTq": np.ascontiguousarray(etq),
            "wpack": wpack,
            "bpack": bpack,
            "thresh": thresh[parity],
        })

    res = run_bass_kernel_spmd(nc, in_maps, list(range(NCORES)))

    out = np.empty((B, S, H), F32)
    for c in range(NCORES):
        b, parity = c // 2, c % 2
        oc = res.results[c]["out"]
        for s, j in enumerate(BLOCKS[parity]):
            out[b, 512 * j:512 * (j + 1)] = oc[512 * s:512 * (s + 1)]
    return out


# revision 13
# speedup vs baseline: 1.2340x; 1.0023x over previous
"""Causal single-head attention (B=4, S=4096, E=1024, H=128) on 8 TRN2 NeuronCores.

Sharding: 8 cores = 4 batches x 2 sequence shards. Each core handles 4 query
blocks of 512 rows of one batch. Causal work per q-block j is 4*(j+1) k-tiles
(128 keys each); blocks are split {7,5,2,0} / {6,4,3,1} so both shards cost 72
k-tiles, padded to a uniform program of [32,24,16,8] k-tiles per slot so all 8
cores run one SPMD program.

Per core, one interleaved pipeline:
  - K^T/V^T projection blocks (bf16 matmuls, fp32 PSUM) streamed off chunked
    embT DMAs, V transposed on the PE, followed by flash-style attention in
    scores-transposed layout (scores^T = K_tile^T.T @ Q^T), exp on ACT,
    causal masking via fused DVE tensor_paged_mask driven by a tiny per-core
    threshold tensor (no mask DMA), AV accumulated in PSUM.
  - Q^T projections and output epilogues are emitted between attention pairs
    as PE filler work so the tensor engine never idles.
  - Softmax denominator: DVE/GPSIMD accumulate exp partials; tiny
    [128,1]-output matmuls reduce them per 128-column chunk into column form,
    so normalization is a per-partition scalar multiply fused with the
    PSUM->SBUF copy after the PE output transpose.
  - ~3us of tiny warm-up matmuls at t=0 keep the PE p-state ramp warm through
    the first DMA latency.
"""

import numpy as np
import ml_dtypes

import concourse.bacc as bacc
import concourse.bass as bass
import concourse.mybir as mybir
import concourse.tile as tile
from concourse.bass_utils import run_bass_kernel_spmd
from concourse.masks import make_identity

BF16 = ml_dtypes.bfloat16
F32 = np.float32

B, S, E, H = 4, 4096, 1024, 128
NCORES = 8
PROG = [32, 24, 16, 8]                       # program k-tile count per slot
BLOCKS = {0: [7, 5, 2, 0], 1: [6, 4, 3, 1]}  # parity -> owned q-block ids
EC = E // 128                                 # 8 contraction chunks
SB = S // 512                                 # 8 key blocks of 512
QB = 4                                        # q-blocks (slots) per core
QLEN = QB * 512                               # 2048 q rows per core

_CACHE = {}


def _build_program():
    dt = mybir.dt
    nc = bacc.Bacc("TRN2", target_bir_lowering=False, debug=False, num_devices=NCORES)

    embT_d = nc.dram_tensor("embT", [E, S], dt.bfloat16, kind="ExternalInput")
    embTq_d = nc.dram_tensor("embTq", [E, QLEN], dt.bfloat16, kind="ExternalInput")
    wpack_d = nc.dram_tensor("wpack", [128, 3, EC, H], dt.bfloat16, kind="ExternalInput")
    bth_d = nc.dram_tensor("bth", [128, 19], dt.float32, kind="ExternalInput")
    out_d = nc.dram_tensor("out", [QLEN, H], dt.float32, kind="ExternalOutput")

    ident_f = mybir.ActivationFunctionType.Identity
    exp_f = mybir.ActivationFunctionType.Exp

    with tile.TileContext(nc) as tc:
        with tc.tile_pool(name="singles", bufs=1) as singles, \
             tc.tile_pool(name="etp", bufs=3) as etp, \
             tc.tile_pool(name="qetp", bufs=2) as qetp, \
             tc.tile_pool(name="vtp", bufs=2) as vtp, \
             tc.tile_pool(name="ptp", bufs=3) as ptp, \
             tc.tile_pool(name="paccp", bufs=2) as paccp, \
             tc.tile_pool(name="ocp", bufs=2) as ocp, \
             tc.tile_pool(name="posp", bufs=2) as posp, \
             tc.tile_pool(name="recp", bufs=2) as recp, \
             tc.tile_pool(name="psp", bufs=2, space="PSUM") as psp, \
             tc.tile_pool(name="pop", bufs=2, space="PSUM") as pop, \
             tc.tile_pool(name="auxp", bufs=2, space="PSUM") as auxp:

            # ---- constants / weights (front of the SP DMA queue) ----
            wpack = singles.tile([128, 3, EC, H], dt.bfloat16, tag="wpack")
            nc.sync.dma_start(out=wpack[:, 0, :, :], in_=wpack_d.ap()[:, 0])
            bth = singles.tile([128, 19], dt.float32, tag="bth")
            nc.sync.dma_start(out=bth[:, :], in_=bth_d.ap())
            bp = bth[:, 0:3]
            th = bth[:, 3:19]
            nc.sync.dma_start(out=wpack[:, 1, :, :], in_=wpack_d.ap()[:, 1])

            # warm-up source available almost immediately (DVE memset)
            wsrc = singles.tile([128, 64], dt.bfloat16, tag="wsrc")
            nc.vector.memset(wsrc[:, :], 0.5)
            identb = singles.tile([128, 128], dt.bfloat16, tag="identb")
            make_identity(nc, identb[:, :])
            identf = singles.tile([128, 128], dt.float32, tag="identf")
            make_identity(nc, identf[:, :])
            ones_f = singles.tile([128, 1], dt.float32, tag="ones_f")
            nc.vector.memset(ones_f[:, :], 1.0)
            ones_r = singles.tile([128, 1], dt.float32r, tag="ones_r")
            nc.vector.tensor_copy(ones_r[:, :], ones_f[:, :])
            # paged-mask offsets: offs[p, j, q] = q + 1 (fp16 keeps DVE 2x mode)
            offs = singles.tile([128, 2, 512], dt.float16, tag="offs")
            nc.gpsimd.iota(offs[:, :, :], pattern=[[0, 2], [1, 512]], base=1,
                           channel_multiplier=0, allow_small_or_imprecise_dtypes=True)

            # ---- PE warm-up: keep the p-state ramp alive through DMA latency ----
            for i in range(75):
                wm = auxp.tile([128, 64], dt.float32, tag="aux", name=f"wm{i}")
                nc.tensor.matmul(wm[:64, :], lhsT=wsrc[:, :], rhs=wsrc[:, :],
                                 start=True, stop=True)

            kTs = [singles.tile([128, 512], dt.bfloat16, tag=f"kT{i}", name=f"kT{i}")
                   for i in range(SB)]
            vts = [singles.tile([128, 128], dt.bfloat16, tag=f"v{i}", name=f"v{i}")
                   for i in range(4 * SB)]
            qTs = [singles.tile([128, 512], dt.bfloat16, tag=f"qT{i}", name=f"qT{i}")
                   for i in range(QB)]

            pending_tr = []  # deferred V-transposes: (st, vT tile, u)

            def flush_tr(n=1000):
                # one PE transpose + DVE copy per call site; spaced so the
                # 2-slot aux rotation never stalls the PE on the DVE copy
                for _ in range(min(n, len(pending_tr))):
                    st, vT, u = pending_tr.pop(0)
                    tp = auxp.tile([128, 128], dt.bfloat16, tag="aux", name=f"tp{st}")
                    nc.tensor.transpose(tp[:, :], vT[:, 128 * u:128 * (u + 1)],
                                        identb[:, :])
                    nc.vector.tensor_copy(vts[st][:, :], tp[:, :])

            def kv_block(sb):
                et = etp.tile([128, EC, 512], dt.bfloat16, tag="et", name=f"et{sb}")
                for c2 in range(EC // 2):
                    nc.sync.dma_start(
                        out=et[:, 2 * c2:2 * c2 + 2, :],
                        in_=embT_d.ap()
                        .rearrange("(cc c p) s -> p cc c s", c=2, p=128)[:, c2, :, 512 * sb:512 * (sb + 1)],
                    )
                ps = psp.tile([128, 2, 512], dt.float32, tag="ps", name=f"pskv{sb}")
                for c in range(EC):
                    nc.tensor.matmul(ps[:, 0, :], lhsT=wpack[:, 0, c, :], rhs=et[:, c, :],
                                     start=(c == 0), stop=(c == EC - 1))
                    nc.tensor.matmul(ps[:, 1, :], lhsT=wpack[:, 1, c, :], rhs=et[:, c, :],
                                     start=(c == 0), stop=(c == EC - 1))
                    if c >= 3:
                        flush_tr(1)  # previous block's V transposes, one per chunk
                nc.scalar.activation(kTs[sb][:, :], ps[:, 0, :], ident_f, bias=bp[:, 0:1])
                vT = vtp.tile([128, 512], dt.bfloat16, tag="vT", name=f"vT{sb}")
                nc.scalar.activation(vT[:, :], ps[:, 1, :], ident_f, bias=bp[:, 1:2])
                pending_tr.extend((4 * sb + u, vT, u) for u in range(4))

            # filler units: emitted between attention pairs to keep PE dense
            fillers = []

            def emit_fillers(n):
                for _ in range(n):
                    if fillers:
                        fillers.pop(0)()

            def qproj(s):
                qet = qetp.tile([128, EC, 512], dt.bfloat16, tag="qet", name=f"qet{s}")
                for c2 in range(EC // 2):
                    nc.sync.dma_start(
                        out=qet[:, 2 * c2:2 * c2 + 2, :],
                        in_=embTq_d.ap()
                        .rearrange("(cc c p) s -> p cc c s", c=2, p=128)[:, c2, :, 512 * s:512 * (s + 1)],
                    )
                ps = psp.tile([128, 2, 512], dt.float32, tag="ps", name=f"psq{s}")
                for c in range(EC):
                    nc.tensor.matmul(ps[:, 0, :], lhsT=wpack[:, 2, c, :],
                                     rhs=qet[:, c, :],
                                     start=(c == 0), stop=(c == EC - 1))
                nc.scalar.activation(qTs[s][:, :], ps[:, 0, :], ident_f,
                                     bias=bp[:, 2:3])

            def att_slot(s):
                Wp = PROG[s]
                NP = Wp // 2
                po = pop.tile([128, 512], dt.float32, tag="po", name=f"po{s}")
                pacc_a = paccp.tile([128, 512], dt.float32r, tag="pacc_a", name=f"pacc_a{s}")
                pacc_b = paccp.tile([128, 512], dt.float32r, tag="pacc_b", name=f"pacc_b{s}")
                pts = {}

                def sc(p):
                    ps = psp.tile([128, 2, 512], dt.float32, tag="ps", name=f"ps{s}_{p}")
                    for h2 in (0, 1):
                        t = 2 * p + h2
                        nc.tensor.matmul(ps[:, h2, :],
                                         lhsT=kTs[t // 4][:, 128 * (t % 4):128 * (t % 4 + 1)],
                                         rhs=qTs[s][:, :], start=True, stop=True)
                    pt = ptp.tile([128, 2, 512], dt.bfloat16, tag="pt", name=f"pt{s}_{p}")
                    nc.scalar.activation(pt[:, :, :], ps[:, :, :], exp_f)
                    pp = p - (NP - 4)
                    if pp >= 0:
                        # fused causal mask: keep iff th[k] + 128*sub < q+1
                        nc.vector.tensor_paged_mask(
                            pt[:, :, :], pt[:, :, :],
                            partition_indices=th[:, 4 * s + pp:4 * s + pp + 1],
                            partition_step=128.0,
                            mask_offsets=offs[:, :, :],
                        )
                    if p == 0:
                        nc.vector.tensor_copy(pacc_a[:, :], pt[:, 0, :])
                        nc.gpsimd.tensor_copy(pacc_b[:, :], pt[:, 1, :])
                    else:
                        nc.vector.tensor_add(pacc_a[:, :], pacc_a[:, :], pt[:, 0, :])
                        nc.gpsimd.tensor_add(pacc_b[:, :], pacc_b[:, :], pt[:, 1, :])
                    pts[p] = pt

                def av(p):
                    pt = pts.pop(p)
                    for h2 in (0, 1):
                        t = 2 * p + h2
                        nc.tensor.matmul(po[:, :], lhsT=vts[t][:, :], rhs=pt[:, h2, :],
                                         start=(t == 0), stop=(t == Wp - 1))

                # software pipeline: scores run 2 pairs ahead of the AV matmuls
                for p in range(NP + 2):
                    if p < NP:
                        sc(p)
                    if p >= 2:
                        av(p - 2)
                        emit_fillers(1)
                # ---- epilogue head: column-form denominator (inline, cheap) ----
                lc = auxp.tile([128, 4], dt.float32, tag="aux", name=f"lc{s}")
                for u in range(4):
                    nc.tensor.matmul(lc[:, u:u + 1], lhsT=pacc_a[:, 128 * u:128 * (u + 1)],
                                     rhs=ones_r[:, :], start=True, stop=False)
                    nc.tensor.matmul(lc[:, u:u + 1], lhsT=pacc_b[:, 128 * u:128 * (u + 1)],
                                     rhs=ones_r[:, :], start=False, stop=True)
                rec = recp.tile([128, 4], dt.float32, tag="rec", name=f"rec{s}")
                nc.vector.reciprocal(rec[:, :], lc[:, :])
                pos = posp.tile([128, 512], dt.float32, tag="pos", name=f"pos{s}")
                nc.vector.tensor_copy(pos[:, :], po[:, :])
                # ---- epilogue tail: transpose + normalize-on-copy, as fillers ----
                oc = ocp.tile([128, 4, 128], dt.float32, tag="oc", name=f"oc{s}")

                def tr_piece(u):
                    def emit():
                        tro = auxp.tile([128, 128], dt.float32, tag="aux",
                                        name=f"tro{s}_{u}")
                        nc.tensor.transpose(tro[:, :], pos[:, 128 * u:128 * (u + 1)],
                                            identf[:, :])
                        nc.vector.tensor_scalar_mul(oc[:, u, :], tro[:, :], rec[:, u:u + 1])
                        nc.sync.dma_start(
                            out=out_d.ap()[512 * s + 128 * u:512 * s + 128 * (u + 1), :],
                            in_=oc[:, u, :],
                        )
                    return emit

                fillers.extend(tr_piece(u) for u in range(4))

            # ---- pipeline ----
            for sb in range(SB):
                kv_block(sb)
                if sb == 3:  # Q weights, needed from qproj(3) at ~30us
                    nc.sync.dma_start(out=wpack[:, 2, :, :], in_=wpack_d.ap()[:, 2])
            qproj(3)
            flush_tr()
            att_slot(3)
            qproj(2)
            att_slot(2)
            qproj(1)
            att_slot(1)
            qproj(0)
            att_slot(0)
            emit_fillers(len(fillers))

    nc.compile()
    return nc


def _build_thresh(parity):
    t = np.zeros((128, 16), np.float32)
    kk = np.arange(128, dtype=np.float32)
    for s, j in enumerate(BLOCKS[parity]):
        Wp, Wa = PROG[s], 4 * (j + 1)
        for pp in range(4):
            t0 = Wp - 8 + 2 * pp
            d0 = t0 - (Wa - 4)
            t[:, 4 * s + pp] = kk + 128.0 * d0
    return t


def kernel(embds, Wq, bq, Wk, bk, Wv, bv):
    embds = np.asarray(embds, F32)
    Wq = np.asarray(Wq, F32); bq = np.asarray(bq, F32)
    Wk = np.asarray(Wk, F32); bk = np.asarray(bk, F32)
    Wv = np.asarray(Wv, F32); bv = np.asarray(bv, F32)

    if "nc" not in _CACHE:
        _CACHE["nc"] = _build_program()
    nc = _CACHE["nc"]

    scale = F32(1.0 / np.sqrt(H))

    def to_lhsT(w):
        return np.ascontiguousarray(
            w.astype(BF16).reshape(EC, 128, H).transpose(1, 0, 2))

    wpack = np.ascontiguousarray(
        np.stack([to_lhsT(Wk), to_lhsT(Wv), to_lhsT(Wq * scale)], axis=1))
    bpack = np.stack([bk, bv, bq * scale], axis=1).astype(F32)
    bth = {p: np.ascontiguousarray(
        np.concatenate([bpack, _build_thresh(p)], axis=1)) for p in (0, 1)}

    embT = {b: np.ascontiguousarray(embds[b].T).astype(BF16) for b in range(B)}

    in_maps = []
    for c in range(NCORES):
        b, parity = c // 2, c % 2
        et = embT[b]
        etq = np.concatenate([et[:, 512 * j:512 * (j + 1)] for j in BLOCKS[parity]], axis=1)
        in_maps.append({
            "embT": et,
            "embTq": np.ascontiguousarray(etq),
            "wpack": wpack,
            "bth": bth[parity],
        })

    res = run_bass_kernel_spmd(nc, in_maps, list(range(NCORES)))

    out = np.empty((B, S, H), F32)
    for c in range(NCORES):
        b, parity = c // 2, c % 2
        oc = res.results[c]["out"]
        for s, j in enumerate(BLOCKS[parity]):
            out[b, 512 * j:512 * (j + 1)] = oc[512 * s:512 * (s + 1)]
    return out
